# revision 17
# baseline (speedup 1.0000x reference)
"""Trainium2 Bass kernel v4 for nn_ReaReaConv (GCN-style message passing with
dynamic edge gating).

Math (per batch b):
    deg[n]   = in-degree(n) + 1 (self loop);  dis = rsqrt(deg)
    f_e      = keep*fdo + (1-keep)*(1-fdo), keep = sigmoid(2*flux[src]*flux[tgt])
    out[t]   = dis_t * ( T[t] @ Wc^T + V_b[t] @ (Wd-Wc)^T ) + bias
    T[t]     = sum_{e->t} dis_src * x[src_e]          (self loop: f=0 edge)
    V_b[t]   = sum_{e->t} dis_src * f_be * x[src_e]

v4 design:
  * The stacked one-hot rhs (values dis_src, dis_src*f0, dis_src*f1 at the
    edge's target-local column) is built by GPSIMD local_scatter directly in
    (chunk, q, t)-contiguous layout (~0.98 ns/elem + 210 ns/call measured);
    no DVE elementwise in the hot loop at all.
  * Per chunk: ONE stationary load (the host-gathered x rows, slot-major)
    and ONE 3*T-column contiguous moving pass accumulates T, V0, V1 into
    PSUM (33-50 ns/chunk measured when the PE stays warm). V_b's valid
    feature rows are batch b's half; the other half is junk, never read.
  * Epilogue: T/V copied (ACT) into SBUF buffers whose column == local node
    id, then per-128-node-window matmuls with Wc^T / (Wd-Wc)^T, ACT
    copy-with-scale (dis_tgt is per-partition there), DMA out.
"""

from dataclasses import dataclass

import numpy as np

N_NODES = 50000
N_EDGES = 1600000
BATCH = 2
C = 64
N_CORES = 8
TILE = 24            # targets per tile (one-hot width)
SPAN = 4             # tiles per psum/scatter span
CHUNK = 128          # edges per matmul chunk (PE contraction)
LCMAX = 28           # max chunks per scatter call (3*TILE*LCMAX <= 2046)
WIN = 128            # nodes per epilogue window
SELF_FLUX = 30.0     # sigmoid(2*30*30)==1.0 -> f==0 for self-loop edges


@dataclass(frozen=True)
class Cfg:
    n_nodes: int
    n_cores: int
    tile: int
    cts: tuple          # per-tile-position chunk counts (shared across cores)
    has_bias: bool = True

    @property
    def npc(self):
        return self.n_nodes // self.n_cores

    @property
    def ntl(self):      # tiles per core
        return -(-self.npc // self.tile)

    @property
    def sct(self):      # total chunks per core
        return sum(self.cts)

    @property
    def nwin(self):     # epilogue windows per core
        return -(-self.npc // WIN)

    @property
    def spans(self):
        """[(t0, t1, c0, c1)] tile/chunk-col ranges per span."""
        out = []
        offs = np.concatenate([[0], np.cumsum(self.cts)])
        for t0 in range(0, self.ntl, SPAN):
            t1 = min(t0 + SPAN, self.ntl)
            out.append((t0, t1, int(offs[t0]), int(offs[t1])))
        return out

    @property
    def calls(self):
        """Scatter calls: [(span_i, ck0, ck1, io, ni)]; io/ni index the
        padded idx array (ni even)."""
        out = []
        io = 0
        for si, (t0, t1, c0, c1) in enumerate(self.spans):
            ck = c0
            while ck < c1:
                ck1 = min(ck + LCMAX, c1)
                ni = -(-(3 * (ck1 - ck)) // 2) * 2
                out.append((si, ck, ck1, io, ni))
                io += ni
                ck = ck1
        return tuple(out)

    @property
    def icols(self):
        return self.calls[-1][3] + self.calls[-1][4]


# -------------------- host prep (indices / layout only) --------------------

def _edge_meta(x, edge_index, f_disc_orig, fluxes, n):
    """Global sorted-by-target edge arrays + x pack table. Indexing only."""
    src0 = np.asarray(edge_index[0]).astype(np.int64)
    tgt0 = np.asarray(edge_index[1]).astype(np.int64)
    x = np.asarray(x, np.float32)
    fdo = np.asarray(f_disc_orig, np.float32)
    fluxes = np.asarray(fluxes, np.float32)

    deg = (np.bincount(tgt0, minlength=n) + 1).astype(np.float32)

    loops = np.arange(n, dtype=np.int64)
    src_all = np.concatenate([src0, loops])
    tgt_all = np.concatenate([tgt0, loops])
    sf = np.full(n, SELF_FLUX, np.float32)
    per_edge_all = np.stack([
        np.concatenate([fdo, np.zeros(n, np.float32)]),
        np.concatenate([fluxes[0][src0], sf]),
        np.concatenate([fluxes[1][src0], sf]),
        np.concatenate([fluxes[0][tgt0], sf]),
        np.concatenate([fluxes[1][tgt0], sf]),
        deg[src_all],
    ])  # [6, E+N]: fdo, fs0, fs1, ft0, ft1, degs

    perm = np.argsort(tgt_all, kind="stable")
    src_s = src_all[perm]
    tgt_s = tgt_all[perm]
    pe_s = per_edge_all[:, perm]

    import ml_dtypes
    xpack = np.concatenate([x[0], x[1]], axis=1).astype(
        ml_dtypes.bfloat16)  # [n, 2C] bf16 slot-table source
    return src_s, tgt_s, pe_s, deg, xpack


def _chunk_counts(tgt_s, cfg_tile, n, n_cores):
    """Per-tile-position chunk counts, max over cores (SPMD needs them equal)."""
    npc = n // n_cores
    ntl = -(-npc // cfg_tile)
    cts = np.zeros(ntl, np.int64)
    for core in range(n_cores):
        base = core * npc
        for tt in range(ntl):
            t0 = base + tt * cfg_tile
            t1 = min(t0 + cfg_tile, base + npc)
            s = np.searchsorted(tgt_s, t0)
            e = np.searchsorted(tgt_s, t1)
            cts[tt] = max(cts[tt], -(-(e - s) // CHUNK))
    return tuple(int(c) for c in np.maximum(cts, 1))


def prep_core(core, cfg: Cfg, src_s, tgt_s, pe_s, deg, xpack):
    """Build one core's dense input tensors. Indexing/layout only."""
    T, ntl, sct = cfg.tile, cfg.ntl, cfg.sct
    npc = cfg.npc
    base = core * npc
    W = sct * CHUNK

    ids = np.zeros(W, np.int64)          # slot -> source node (pad: 0)
    tl = np.full(W, -1, np.int64)        # slot -> local target (pad: -1)
    pe = np.zeros((6, W), np.float32)
    pe[5] = 1.0                          # pad deg_src = 1

    off = 0
    for tt in range(ntl):
        t0 = base + tt * T
        t1 = min(t0 + T, base + npc)
        s = np.searchsorted(tgt_s, t0)
        e = np.searchsorted(tgt_s, t1)
        ct = cfg.cts[tt]
        assert e - s <= ct * CHUNK
        ids[off:off + (e - s)] = src_s[s:e]
        tl[off:off + (e - s)] = tgt_s[s:e] - t0
        pe[:, off:off + (e - s)] = pe_s[:, s:e]
        off += ct * CHUNK
    assert off == W

    degown = np.ones((128, cfg.nwin), np.float32)
    for w in range(cfg.nwin):
        n0 = base + w * WIN
        n1 = min(n0 + WIN, base + npc)
        degown[:n1 - n0, w] = deg[n0:n1]

    # chunk-transposed views: column (p, c) = slot c*128+p
    def ctr(a):
        return np.ascontiguousarray(a.reshape(sct, CHUNK).T)

    # scatter idx table [128, icols] int16, per-call sections:
    # j = 3*cl + q -> idx = cl*3T + q*T + tl  (pad slots/cols: -1)
    tlc = ctr(tl)  # [128, sct]
    idxs = np.full((128, cfg.icols), -1, np.int16)
    for si, ck0, ck1, io, ni in cfg.calls:
        for cl in range(ck1 - ck0):
            t_loc = tlc[:, ck0 + cl]
            valid = t_loc >= 0
            for q in range(3):
                idxs[:, io + 3 * cl + q] = np.where(
                    valid, cl * 3 * T + q * T + t_loc, -1).astype(np.int16)

    # dense x table [128, sct*128]: slot (c,p) row occupies cols c*128..+128
    # on partition p
    xg = np.ascontiguousarray(
        xpack[ids].reshape(sct, CHUNK, 2 * C).transpose(1, 0, 2)
        .reshape(CHUNK, W))

    import ml_dtypes
    bf = ml_dtypes.bfloat16
    d = {
        "xg": xg,
        "idxs": idxs,
        "fdo": ctr(pe[0]).astype(bf), "fs0": ctr(pe[1]).astype(bf),
        "fs1": ctr(pe[2]).astype(bf), "ft0": ctr(pe[3]).astype(bf),
        "ft1": ctr(pe[4]).astype(bf), "degs": ctr(pe[5]).astype(bf),
        "degown": degown,
    }
    return d


# -------------------- device program --------------------

def build_nc(cfg: Cfg):
    import concourse.bass as bass  # noqa: F401
    import concourse.tile as tile
    from concourse import bacc, mybir, library_config

    dt = mybir.dt
    act = mybir.ActivationFunctionType
    alu = mybir.AluOpType

    T, ntl, sct = cfg.tile, cfg.ntl, cfg.sct
    spans = cfg.spans
    nwin = cfg.nwin
    ncols = max(ntl * T, nwin * WIN)

    nc = bacc.Bacc("TRN2", target_bir_lowering=False, debug=False)

    xg_d = nc.dram_tensor("xg", [128, sct * CHUNK], dt.bfloat16,
                          kind="ExternalInput")
    idxs_d = nc.dram_tensor("idxs", [128, cfg.icols], dt.int16,
                            kind="ExternalInput")
    fdo_d = nc.dram_tensor("fdo", [128, sct], dt.bfloat16,
                           kind="ExternalInput")
    fs0_d = nc.dram_tensor("fs0", [128, sct], dt.bfloat16,
                           kind="ExternalInput")
    fs1_d = nc.dram_tensor("fs1", [128, sct], dt.bfloat16,
                           kind="ExternalInput")
    ft0_d = nc.dram_tensor("ft0", [128, sct], dt.bfloat16,
                           kind="ExternalInput")
    ft1_d = nc.dram_tensor("ft1", [128, sct], dt.bfloat16,
                           kind="ExternalInput")
    degs_d = nc.dram_tensor("degs", [128, sct], dt.bfloat16,
                            kind="ExternalInput")
    degown_d = nc.dram_tensor("degown", [128, nwin], dt.float32,
                              kind="ExternalInput")
    wct_d = nc.dram_tensor("wct2", [128, C], dt.float32, kind="ExternalInput")
    wdt_d = nc.dram_tensor("wdt2", [128, C], dt.float32, kind="ExternalInput")
    bias_d = nc.dram_tensor("biasr", [128, C], dt.float32,
                            kind="ExternalInput")
    out0 = nc.dram_tensor("out0", [nwin * WIN, C], dt.float32,
                          kind="ExternalOutput")
    out1 = nc.dram_tensor("out1", [nwin * WIN, C], dt.float32,
                          kind="ExternalOutput")
    outs = [out0, out1]

    with tile.TileContext(nc) as tc:
        nc.gpsimd.load_library(library_config.local_scatter)
        with (
            tc.tile_pool(name="const", bufs=1) as constp,
            tc.tile_pool(name="res", bufs=1) as resp,
        ):
            biasf_sb = constp.tile([128, C], dt.float32)
            nc.sync.dma_start(biasf_sb[:], bias_d[:, :])
            wctf_sb = constp.tile([128, C], dt.float32)
            nc.sync.dma_start(wctf_sb[:], wct_d[:, :])
            wdtf_sb = constp.tile([128, C], dt.float32)
            nc.sync.dma_start(wdtf_sb[:], wdt_d[:, :])
            # bf16 Wc^T and (Wd-Wc)^T
            wct_sb = constp.tile([128, C], dt.bfloat16)
            nc.vector.tensor_copy(out=wct_sb[:], in_=wctf_sb[:])
            wdl_sb = constp.tile([128, C], dt.bfloat16)
            nc.vector.tensor_tensor(wdtf_sb[:], wdtf_sb[:], wctf_sb[:],
                                    alu.subtract)
            nc.vector.tensor_copy(out=wdl_sb[:], in_=wdtf_sb[:])

            idxs_sb = resp.tile([128, cfg.icols], dt.int16)
            nc.sync.dma_start(idxs_sb[:], idxs_d[:, :])
            # scatter data, interleaved per chunk: col 3c+q = w_q[:, c]
            gall_sb = resp.tile([128, 3 * sct + 2], dt.bfloat16)
            nc.vector.memset(gall_sb[:, 3 * sct:], 0)
            gall3 = gall_sb[:, :3 * sct].rearrange("p (c q) -> p c q", q=3)

            disown_sb = resp.tile([128, nwin], dt.float32)
            nc.sync.dma_start(disown_sb[:], degown_d[:, :])
            nc.vector.reciprocal_approx_fast(disown_sb[:], disown_sb[:])
            nc.scalar.activation(disown_sb[:], disown_sb[:], act.Sqrt)

            # accumulation buffers: col j == local node j
            um_sb = resp.tile([128, ncols], dt.bfloat16)
            v0_sb = resp.tile([128, ncols], dt.bfloat16)
            v1_sb = resp.tile([128, ncols], dt.bfloat16)
            vq_sb = [um_sb, v0_sb, v1_sb]
            if ncols > ntl * T:
                for q in range(3):
                    nc.vector.memset(vq_sb[q][:, ntl * T:], 0)

            # ---- main pools (xg prefetch must precede prepass DMAs) ----
            span_calls = {}
            for si, ck0, ck1, io, ni in cfg.calls:
                span_calls.setdefault(si, []).append((ck0, ck1, io, ni))

            with (
                tc.tile_pool(name="xgp", bufs=4) as xgp,
                tc.tile_pool(name="ohxp", bufs=2) as ohxp,
                tc.tile_pool(name="ps_tv", bufs=3, space="PSUM") as pstv,
                tc.tile_pool(name="ps_o", bufs=2, space="PSUM") as pso,
                tc.tile_pool(name="outp", bufs=4) as outsp,
            ):
                offs = np.concatenate([[0], np.cumsum(cfg.cts)])
                PF = 4
                xg_pre = {}

                def issue_xg(si):
                    if si >= len(spans):
                        return
                    _, _, c0, c1 = spans[si]
                    x = xgp.tile([128, (c1 - c0) * CHUNK], dt.bfloat16,
                                 tag="xg")
                    nc.sync.dma_start(x[:], xg_d[:, c0 * CHUNK:c1 * CHUNK])
                    xg_pre[si] = x

                for si in range(PF):
                    issue_xg(si)

                # ---- prepass: g, g*f0, g*f1 -> gall (DVE + ACT + GPSIMD) --
                # geometric segments: small early ones unblock the first
                # spans' scatters quickly
                segb = [0]
                stepw = max(2 * LCMAX, sct // 16)
                while segb[-1] < sct:
                    segb.append(min(sct, segb[-1] + stepw))
                    stepw = min(2 * stepw, (sct * 2) // 8)
                nseg = len(segb) - 1
                with tc.tile_pool(name="pp", bufs=2) as ppp:
                    for i in range(nseg):
                        sl = slice(segb[i], segb[i + 1])
                        w = segb[i + 1] - segb[i]
                        gh = ppp.tile([128, w], dt.bfloat16, tag="gh")
                        nc.sync.dma_start(gh[:], degs_d[:, sl])
                        g = ppp.tile([128, w], dt.float32, tag="g")
                        nc.vector.tensor_copy(out=g[:], in_=gh[:])
                        nc.vector.reciprocal_approx_fast(g[:], g[:])
                        nc.scalar.activation(g[:], g[:], act.Sqrt)
                        nc.vector.tensor_copy(out=gall3[:, sl, 0], in_=g[:])
                        fdoh = ppp.tile([128, w], dt.bfloat16, tag="fdoh")
                        nc.sync.dma_start(fdoh[:], fdo_d[:, sl])
                        # gc1 = g*(2*fdo-1), gc0 = g*(1-fdo);
                        # gf_b = sigmoid(2*fs*ft)*gc1 + gc0
                        c1 = ppp.tile([128, w], dt.float32, tag="c1")
                        nc.vector.tensor_scalar(
                            c1[:], fdoh[:], 2.0, -1.0, alu.mult, alu.add)
                        nc.vector.tensor_mul(c1[:], c1[:], g[:])
                        c0 = ppp.tile([128, w], dt.float32, tag="c0")
                        nc.vector.tensor_scalar(
                            c0[:], fdoh[:], -1.0, 1.0, alu.mult, alu.add)
                        nc.vector.tensor_mul(c0[:], c0[:], g[:])
                        for b, (fsd, ftd) in enumerate(
                                ((fs0_d, ft0_d), (fs1_d, ft1_d))):
                            fs = ppp.tile([128, w], dt.bfloat16, tag=f"fs{b}")
                            ft = ppp.tile([128, w], dt.bfloat16, tag=f"ft{b}")
                            nc.sync.dma_start(fs[:], fsd[:, sl])
                            nc.sync.dma_start(ft[:], ftd[:, sl])
                            z = ppp.tile([128, w], dt.float32, tag=f"z{b}")
                            nc.gpsimd.tensor_mul(z[:], fs[:], ft[:])
                            nc.scalar.activation(z[:], z[:], act.Sigmoid,
                                                 scale=2.0)
                            nc.vector.tensor_mul(z[:], z[:], c1[:])
                            nc.vector.tensor_tensor(
                                gall3[:, sl, 1 + b], z[:], c0[:], alu.add)

                def do_span(si):
                    t0, t1, c0, c1 = spans[si]
                    L = c1 - c0
                    gs = t1 - t0

                    xgs = xg_pre.pop(si)
                    issue_xg(si + PF)

                    ohx = ohxp.tile([128, 3 * T * L], dt.bfloat16, tag="ohx")
                    for ck0, ck1, io, ni in span_calls[si]:
                        lk = ck1 - ck0
                        d0 = (ck0 - c0) * 3 * T
                        nc.gpsimd.local_scatter(
                            ohx[:, d0:d0 + lk * 3 * T],
                            gall_sb[:, 3 * ck0:3 * ck0 + ni],
                            idxs_sb[:, io:io + ni],
                            channels=128, num_elems=lk * 3 * T, num_idxs=ni)

                    ps = pstv.tile([128, gs * 3 * T], dt.float32, tag="ps")
                    for tt in range(t0, t1):
                        g3 = (tt - t0) * 3 * T
                        ct = cfg.cts[tt]
                        first = int(offs[tt]) - c0
                        for k in range(ct):
                            sc = first + k
                            nc.tensor.matmul(
                                out=ps[:, g3:g3 + 3 * T],
                                lhsT=xgs[:, sc * CHUNK:(sc + 1) * CHUNK],
                                rhs=ohx[:, sc * 3 * T:(sc + 1) * 3 * T],
                                start=(k == 0), stop=(k == ct - 1),
                            )
                    # psum -> global accum buffers (cast bf16), on ACT
                    ps4 = ps[:].rearrange("p (g q t) -> p g q t", q=3, t=T)
                    for q in range(3):
                        nc.scalar.activation(
                            vq_sb[q][:, t0 * T:t1 * T]
                            .rearrange("p (g t) -> p g t", t=T),
                            ps4[:, :, q, :], act.Copy)

                def do_window(w, bi):
                    rows = slice(C * bi, C * bi + C)
                    ws = slice(w * WIN, (w + 1) * WIN)
                    vb = vq_sb[1 + bi]
                    op = pso.tile([WIN, C], dt.float32, tag=f"op{bi}")
                    nc.tensor.matmul(out=op[:], lhsT=um_sb[rows, ws],
                                     rhs=wct_sb[rows, :],
                                     start=True, stop=False)
                    nc.tensor.matmul(out=op[:], lhsT=vb[rows, ws],
                                     rhs=wdl_sb[rows, :],
                                     start=False, stop=True)
                    o_sb = outsp.tile([WIN, C], dt.float32, tag=f"os{bi}")
                    nc.scalar.activation(o_sb[:], op[:], act.Copy,
                                         scale=disown_sb[:WIN, w:w + 1])
                    if cfg.has_bias:
                        nc.vector.tensor_add(o_sb[:], o_sb[:],
                                             biasf_sb[:WIN, :])
                    nc.sync.dma_start(outs[bi][ws, :], o_sb[:])

                # interleave: issue epilogue windows as their tiles complete
                nwin_done = 0
                for si in range(len(spans)):
                    do_span(si)
                    ready_nodes = spans[si][1] * T
                    while (nwin_done < nwin
                           and (nwin_done + 1) * WIN <= ready_nodes):
                        for bi in range(2):
                            do_window(nwin_done, bi)
                        nwin_done += 1
                while nwin_done < nwin:
                    for bi in range(2):
                        do_window(nwin_done, bi)
                    nwin_done += 1

    nc.compile()
    return nc


def _shared_weights(W_conc, W_disc, bias):
    wct2 = np.zeros((128, C), np.float32)
    wdt2 = np.zeros((128, C), np.float32)
    wct2[:C] = np.asarray(W_conc, np.float32).T  # WcT[i, o] = Wc[o, i]
    wct2[C:] = wct2[:C]
    wdt2[:C] = np.asarray(W_disc, np.float32).T
    wdt2[C:] = wdt2[:C]
    biasr = np.tile(np.asarray(bias, np.float32)[None, :], (128, 1))
    return wct2, wdt2, biasr


_NC_CACHE = {}


def _run(inputs, trace=False):
    from concourse.bass_utils import run_bass_kernel_spmd

    x = np.asarray(inputs["x"], np.float32)
    n = x.shape[1]
    src_s, tgt_s, pe_s, deg, xpack = _edge_meta(
        x, inputs["edge_index"], inputs["f_disc_orig"], inputs["fluxes"], n)
    cts = _chunk_counts(tgt_s, TILE, n, N_CORES)
    cfg = Cfg(n_nodes=n, n_cores=N_CORES, tile=TILE, cts=cts,
              has_bias=bool(np.any(np.asarray(inputs["bias"]))))

    wct2, wdt2, biasr = _shared_weights(
        inputs["W_conc"], inputs["W_disc"], inputs["bias"])

    in_maps = []
    for core in range(cfg.n_cores):
        m = prep_core(core, cfg, src_s, tgt_s, pe_s, deg, xpack)
        m.update(wct2=wct2, wdt2=wdt2, biasr=biasr)
        in_maps.append(m)

    if cfg not in _NC_CACHE:
        _NC_CACHE[cfg] = build_nc(cfg)
    nc = _NC_CACHE[cfg]

    res = run_bass_kernel_spmd(nc, in_maps, list(range(cfg.n_cores)),
                               trace=trace)
    out = np.zeros((BATCH, n, C), np.float32)
    npc = cfg.npc
    for core in range(cfg.n_cores):
        out[0, core * npc:(core + 1) * npc] = res.results[core]["out0"][:npc]
        out[1, core * npc:(core + 1) * npc] = res.results[core]["out1"][:npc]
    return out, res


def kernel(x, edge_index, f_disc_orig, fluxes, W_conc, W_disc, bias):
    out, _ = _run(dict(x=x, edge_index=edge_index, f_disc_orig=f_disc_orig,
                       fluxes=fluxes, W_conc=W_conc, W_disc=W_disc, bias=bias))
    return out


def profile_run(inputs):
    out, res = _run(inputs, trace=True)
    return res.exec_time_ns


# revision 25
# speedup vs baseline: 1.0524x; 1.0524x over previous
"""Trainium2 Bass kernel v5 for nn_ReaReaConv (GCN-style message passing with
dynamic edge gating).

Math (per batch b):
    deg[n]   = in-degree(n) + 1 (self loop);  dis = rsqrt(deg)
    f_e      = keep*fdo + (1-keep)*(1-fdo), keep = sigmoid(2*flux[src]*flux[tgt])
    out[t]   = dis_t * ( T[t] @ Wc^T + V_b[t] @ (Wd-Wc)^T ) + bias
    T[t]     = sum_{e->t} dis_src * x[src_e]          (self loop: f=0 edge)
    V_b[t]   = sum_{e->t} dis_src * f_be * x[src_e]

v5 design (v4 + span-level chunking):
  * Edges are chunked at SPAN granularity (SPAN tiles per span), not per
    tile, killing the per-tile ceil padding of the gathered-x DMA stream
    (~8%). A chunk may cross one tile boundary; each chunk position gets a
    host-computed static window (base tile a_k, width w_k tiles) that is the
    union over all 8 cores, so the SPMD program is shared. PSUM is zeroed
    once per span and all matmuls accumulate with start=False.
  * The stacked one-hot rhs (values dis_src, dis_src*f0, dis_src*f1 at
    column (tile_rel-a_k)*3T + q*T + t_local of the chunk's block) is built
    by GPSIMD local_scatter directly in contiguous layout; no DVE
    elementwise in the hot loop.
  * Per chunk: ONE stationary load (gathered x rows, slot-major) and ONE
    w_k*3T-column contiguous moving pass accumulates T, V0, V1. V_b's valid
    feature rows are batch b's half; the other half is junk, never read.
  * Epilogue: T/V copied (ACT) into SBUF buffers whose column == local node
    id, then per-128-node-window matmuls with Wc^T / (Wd-Wc)^T, ACT
    copy-with-scale (dis_tgt is per-partition there), DMA out.
"""

from dataclasses import dataclass

import numpy as np

N_NODES = 50000
N_EDGES = 1600000
BATCH = 2
C = 64
N_CORES = 8
TILE = 24            # targets per tile (one-hot width)
SPAN = 4             # tiles per psum/scatter span
CHUNK = 128          # edges per matmul chunk (PE contraction)
NEMAX = 2046         # max scatter dst elems per call (uint16 byte offsets)
WIN = 128            # nodes per epilogue window
SELF_FLUX = 30.0     # sigmoid(2*30*30)==1.0 -> f==0 for self-loop edges


@dataclass(frozen=True)
class Cfg:
    n_nodes: int
    n_cores: int
    tile: int
    scs: tuple          # per-span chunk counts (shared across cores)
    wks: tuple          # per span: tuple of (a_k, w_k) per chunk position
    has_bias: bool = True

    @property
    def npc(self):
        return self.n_nodes // self.n_cores

    @property
    def ntl(self):      # tiles per core
        return -(-self.npc // self.tile)

    @property
    def nspan(self):
        return -(-self.ntl // SPAN)

    @property
    def sct(self):      # total chunks per core
        return sum(self.scs)

    @property
    def nwin(self):     # epilogue windows per core
        return -(-self.npc // WIN)

    @property
    def spans(self):
        """[(t0, t1, c0, c1)] tile/chunk-col ranges per span."""
        out = []
        coff = np.concatenate([[0], np.cumsum(self.scs)])
        for si in range(self.nspan):
            t0 = si * SPAN
            t1 = min(t0 + SPAN, self.ntl)
            out.append((t0, t1, int(coff[si]), int(coff[si + 1])))
        return out

    def span_offs(self, si):
        """Per-chunk rhs col offsets within span si's ohx block + total."""
        T3 = 3 * self.tile
        offs = [0]
        for a_k, w_k in self.wks[si]:
            offs.append(offs[-1] + w_k * T3)
        return offs

    @property
    def calls(self):
        """Scatter calls: [(span_i, k0, k1, io, ni)]; io/ni index the padded
        idx array (ni even). dst = ohx[:, offs[k0]:offs[k1]]."""
        out = []
        io = 0
        for si in range(self.nspan):
            offs = self.span_offs(si)
            L = self.scs[si]
            k = 0
            while k < L:
                k1 = k
                while k1 < L and offs[k1 + 1] - offs[k] <= NEMAX:
                    k1 += 1
                assert k1 > k
                ni = -(-(3 * (k1 - k)) // 2) * 2
                out.append((si, k, k1, io, ni))
                io += ni
                k = k1
        return tuple(out)

    @property
    def icols(self):
        return self.calls[-1][3] + self.calls[-1][4]


# -------------------- host prep (indices / layout only) --------------------

def _edge_meta(x, edge_index, f_disc_orig, fluxes, n):
    """Global sorted-by-target edge arrays + x pack table. Indexing only."""
    src0 = np.asarray(edge_index[0]).astype(np.int64)
    tgt0 = np.asarray(edge_index[1]).astype(np.int64)
    x = np.asarray(x, np.float32)
    fdo = np.asarray(f_disc_orig, np.float32)
    fluxes = np.asarray(fluxes, np.float32)

    deg = (np.bincount(tgt0, minlength=n) + 1).astype(np.float32)

    loops = np.arange(n, dtype=np.int64)
    src_all = np.concatenate([src0, loops])
    tgt_all = np.concatenate([tgt0, loops])
    sf = np.full(n, SELF_FLUX, np.float32)
    per_edge_all = np.stack([
        np.concatenate([fdo, np.zeros(n, np.float32)]),
        np.concatenate([fluxes[0][src0], sf]),
        np.concatenate([fluxes[1][src0], sf]),
        np.concatenate([fluxes[0][tgt0], sf]),
        np.concatenate([fluxes[1][tgt0], sf]),
        deg[src_all],
    ])  # [6, E+N]: fdo, fs0, fs1, ft0, ft1, degs

    perm = np.argsort(tgt_all, kind="stable")
    src_s = src_all[perm]
    tgt_s = tgt_all[perm]
    pe_s = per_edge_all[:, perm]

    import ml_dtypes
    xpack = np.concatenate([x[0], x[1]], axis=1).astype(
        ml_dtypes.bfloat16)  # [n, 2C] bf16 slot-table source
    return src_s, tgt_s, pe_s, deg, xpack


def _span_meta(tgt_s, T, n, n_cores):
    """Per-span chunk counts + per-chunk (a_k, w_k) windows, shared across
    cores. Indexing only."""
    npc = n // n_cores
    ntl = -(-npc // T)
    nspan = -(-ntl // SPAN)
    scs = np.zeros(nspan, np.int64)
    for core in range(n_cores):
        base = core * npc
        for si in range(nspan):
            t0 = base + si * SPAN * T
            t1 = min(base + (si + 1) * SPAN * T, base + npc)
            s = np.searchsorted(tgt_s, t0)
            e = np.searchsorted(tgt_s, t1)
            scs[si] = max(scs[si], -(-(e - s) // CHUNK))
    scs = np.maximum(scs, 1)

    wks = []
    for si in range(nspan):
        amin = np.full(scs[si], SPAN, np.int64)
        amax = np.full(scs[si], -1, np.int64)
        for core in range(n_cores):
            base = core * npc
            t0 = base + si * SPAN * T
            t1 = min(base + (si + 1) * SPAN * T, base + npc)
            s = np.searchsorted(tgt_s, t0)
            e = np.searchsorted(tgt_s, t1)
            tl_rel = (tgt_s[s:e] - t0) // T  # tile index within span
            for k in range(-(-(e - s) // CHUNK)):
                seg = tl_rel[k * CHUNK:(k + 1) * CHUNK]
                amin[k] = min(amin[k], seg[0])
                amax[k] = max(amax[k], seg[-1])
        amin = np.minimum(amin, np.maximum(amax, 0))
        span_tiles = min(SPAN, ntl - si * SPAN)
        amax = np.clip(amax, amin, span_tiles - 1)
        wks.append(tuple((int(a), int(b - a + 1))
                         for a, b in zip(amin, amax)))
    return tuple(int(c) for c in scs), tuple(wks)


def prep_core(core, cfg: Cfg, src_s, tgt_s, pe_s, deg, xpack):
    """Build one core's dense input tensors. Indexing/layout only."""
    T, sct = cfg.tile, cfg.sct
    npc = cfg.npc
    base = core * npc
    W = sct * CHUNK

    ids = np.zeros(W, np.int64)          # slot -> source node (pad: 0)
    # per-slot scatter idx value (pad: -1), in span-local chunk blocks
    sidx = np.full(W, -1, np.int64)
    pe = np.zeros((6, W), np.float32)
    pe[5] = 1.0                          # pad deg_src = 1

    for si, (t0, t1, c0, c1) in enumerate(cfg.spans):
        offs = cfg.span_offs(si)
        g0 = base + t0 * T
        g1 = min(base + t1 * T, base + npc)
        s = np.searchsorted(tgt_s, g0)
        e = np.searchsorted(tgt_s, g1)
        ne = e - s
        assert ne <= cfg.scs[si] * CHUNK
        so = c0 * CHUNK
        ids[so:so + ne] = src_s[s:e]
        pe[:, so:so + ne] = pe_s[:, s:e]
        tl_loc = tgt_s[s:e] - g0          # 0 .. span_tiles*T-1
        tile_rel = tl_loc // T
        tin = tl_loc % T
        kk = np.arange(ne) // CHUNK
        a_k = np.array([w[0] for w in cfg.wks[si]], np.int64)
        w_k = np.array([w[1] for w in cfg.wks[si]], np.int64)
        assert np.all(tile_rel >= a_k[kk]), (si, core)
        assert np.all(tile_rel < a_k[kk] + w_k[kk]), (si, core)
        sidx[so:so + ne] = (np.asarray(offs, np.int64)[kk]
                            + (tile_rel - a_k[kk]) * 3 * T + tin)

    degown = np.ones((128, cfg.nwin), np.float32)
    for w in range(cfg.nwin):
        n0 = base + w * WIN
        n1 = min(n0 + WIN, base + npc)
        degown[:n1 - n0, w] = deg[n0:n1]

    # chunk-transposed views: column (p, c) = slot c*128+p
    def ctr(a):
        return np.ascontiguousarray(a.reshape(sct, CHUNK).T)

    # scatter idx table [128, icols] int16 per-call sections (j = 3*cl + q;
    # idx relative to the call's dst slice)
    sidxc = ctr(sidx)  # [128, sct]
    idxs = np.full((128, cfg.icols), -1, np.int16)
    for si, k0, k1, io, ni in cfg.calls:
        offs = cfg.span_offs(si)
        _, _, c0, _ = cfg.spans[si]
        for cl in range(k1 - k0):
            v = sidxc[:, c0 + k0 + cl]
            valid = v >= 0
            for q in range(3):
                idxs[:, io + 3 * cl + q] = np.where(
                    valid, v - offs[k0] + q * T, -1).astype(np.int16)

    # dense x table [128, sct*128]: slot (c,p) row occupies cols c*128..+128
    # on partition p
    xg = np.ascontiguousarray(
        xpack[ids].reshape(sct, CHUNK, 2 * C).transpose(1, 0, 2)
        .reshape(CHUNK, W))

    import ml_dtypes
    bf = ml_dtypes.bfloat16
    d = {
        "xg": xg,
        "idxs": idxs,
        "fdo": ctr(pe[0]).astype(bf), "fs0": ctr(pe[1]).astype(bf),
        "fs1": ctr(pe[2]).astype(bf), "ft0": ctr(pe[3]).astype(bf),
        "ft1": ctr(pe[4]).astype(bf), "degs": ctr(pe[5]).astype(bf),
        "degown": degown,
    }
    return d


# -------------------- device program --------------------

def build_nc(cfg: Cfg):
    import concourse.bass as bass  # noqa: F401
    import concourse.tile as tile
    from concourse import bacc, mybir, library_config

    dt = mybir.dt
    act = mybir.ActivationFunctionType
    alu = mybir.AluOpType

    T, ntl, sct = cfg.tile, cfg.ntl, cfg.sct
    T3 = 3 * T
    spans = cfg.spans
    nwin = cfg.nwin
    ncols = max(ntl * T, nwin * WIN)

    nc = bacc.Bacc("TRN2", target_bir_lowering=False, debug=False)

    xg_d = nc.dram_tensor("xg", [128, sct * CHUNK], dt.bfloat16,
                          kind="ExternalInput")
    idxs_d = nc.dram_tensor("idxs", [128, cfg.icols], dt.int16,
                            kind="ExternalInput")
    fdo_d = nc.dram_tensor("fdo", [128, sct], dt.bfloat16,
                           kind="ExternalInput")
    fs0_d = nc.dram_tensor("fs0", [128, sct], dt.bfloat16,
                           kind="ExternalInput")
    fs1_d = nc.dram_tensor("fs1", [128, sct], dt.bfloat16,
                           kind="ExternalInput")
    ft0_d = nc.dram_tensor("ft0", [128, sct], dt.bfloat16,
                           kind="ExternalInput")
    ft1_d = nc.dram_tensor("ft1", [128, sct], dt.bfloat16,
                           kind="ExternalInput")
    degs_d = nc.dram_tensor("degs", [128, sct], dt.bfloat16,
                            kind="ExternalInput")
    degown_d = nc.dram_tensor("degown", [128, nwin], dt.float32,
                              kind="ExternalInput")
    wct_d = nc.dram_tensor("wct2", [128, C], dt.float32, kind="ExternalInput")
    wdt_d = nc.dram_tensor("wdt2", [128, C], dt.float32, kind="ExternalInput")
    bias_d = nc.dram_tensor("biasr", [128, C], dt.float32,
                            kind="ExternalInput")
    out0 = nc.dram_tensor("out0", [nwin * WIN, C], dt.float32,
                          kind="ExternalOutput")
    out1 = nc.dram_tensor("out1", [nwin * WIN, C], dt.float32,
                          kind="ExternalOutput")
    outs = [out0, out1]

    with tile.TileContext(nc) as tc:
        nc.gpsimd.load_library(library_config.local_scatter)
        with (
            tc.tile_pool(name="const", bufs=1) as constp,
            tc.tile_pool(name="res", bufs=1) as resp,
        ):
            biasf_sb = constp.tile([128, C], dt.float32)
            nc.sync.dma_start(biasf_sb[:], bias_d[:, :])
            wctf_sb = constp.tile([128, C], dt.float32)
            nc.sync.dma_start(wctf_sb[:], wct_d[:, :])
            wdtf_sb = constp.tile([128, C], dt.float32)
            nc.sync.dma_start(wdtf_sb[:], wdt_d[:, :])
            # bf16 Wc^T and (Wd-Wc)^T
            wct_sb = constp.tile([128, C], dt.bfloat16)
            nc.vector.tensor_copy(out=wct_sb[:], in_=wctf_sb[:])
            wdl_sb = constp.tile([128, C], dt.bfloat16)
            nc.vector.tensor_tensor(wdtf_sb[:], wdtf_sb[:], wctf_sb[:],
                                    alu.subtract)
            nc.vector.tensor_copy(out=wdl_sb[:], in_=wdtf_sb[:])
            # all-zero operand for psum-clearing dummy matmuls (PE-ordered)
            zero_sb = constp.tile([128, SPAN * 3 * T], dt.bfloat16)
            nc.vector.memset(zero_sb[:], 0)

            idxs_sb = resp.tile([128, cfg.icols], dt.int16)
            nc.sync.dma_start(idxs_sb[:], idxs_d[:, :])
            # scatter data, interleaved per chunk: col 3c+q = w_q[:, c]
            gall_sb = resp.tile([128, 3 * sct + 2], dt.bfloat16)
            nc.vector.memset(gall_sb[:, 3 * sct:], 0)
            gall3 = gall_sb[:, :3 * sct].rearrange("p (c q) -> p c q", q=3)

            disown_sb = resp.tile([128, nwin], dt.float32)
            nc.sync.dma_start(disown_sb[:], degown_d[:, :])
            nc.vector.reciprocal_approx_fast(disown_sb[:], disown_sb[:])
            nc.scalar.activation(disown_sb[:], disown_sb[:], act.Sqrt)

            # accumulation buffers: col j == local node j
            um_sb = resp.tile([128, ncols], dt.bfloat16)
            v0_sb = resp.tile([128, ncols], dt.bfloat16)
            v1_sb = resp.tile([128, ncols], dt.bfloat16)
            vq_sb = [um_sb, v0_sb, v1_sb]
            if ncols > ntl * T:
                for q in range(3):
                    nc.vector.memset(vq_sb[q][:, ntl * T:], 0)

            # ---- main pools (xg prefetch must precede prepass DMAs) ----
            span_calls = {}
            for si, k0, k1, io, ni in cfg.calls:
                span_calls.setdefault(si, []).append((k0, k1, io, ni))

            with (
                tc.tile_pool(name="xgp", bufs=4) as xgp,
                tc.tile_pool(name="ohxp", bufs=2) as ohxp,
                tc.tile_pool(name="ps_tv", bufs=3, space="PSUM") as pstv,
                tc.tile_pool(name="ps_o", bufs=2, space="PSUM") as pso,
                tc.tile_pool(name="outp", bufs=4) as outsp,
            ):
                PF = 4
                xg_pre = {}

                def issue_xg(si):
                    if si >= len(spans):
                        return
                    _, _, c0, c1 = spans[si]
                    x = xgp.tile([128, (c1 - c0) * CHUNK], dt.bfloat16,
                                 tag="xg")
                    nc.sync.dma_start(x[:], xg_d[:, c0 * CHUNK:c1 * CHUNK])
                    xg_pre[si] = x

                for si in range(PF):
                    issue_xg(si)

                # ---- prepass: g, g*f0, g*f1 -> gall (all-DVE + ACT) ----
                nseg = 8
                segb = [(sct * i) // nseg for i in range(nseg + 1)]
                with tc.tile_pool(name="pp", bufs=2) as ppp:
                    for i in range(nseg):
                        sl = slice(segb[i], segb[i + 1])
                        w = segb[i + 1] - segb[i]
                        gh = ppp.tile([128, w], dt.bfloat16, tag="gh")
                        nc.sync.dma_start(gh[:], degs_d[:, sl])
                        g = ppp.tile([128, w], dt.float32, tag="g")
                        nc.vector.tensor_copy(out=g[:], in_=gh[:])
                        nc.vector.reciprocal_approx_fast(g[:], g[:])
                        nc.scalar.activation(g[:], g[:], act.Sqrt)
                        nc.vector.tensor_copy(out=gall3[:, sl, 0], in_=g[:])
                        fdoh = ppp.tile([128, w], dt.bfloat16, tag="fdoh")
                        nc.sync.dma_start(fdoh[:], fdo_d[:, sl])
                        # gc1 = g*(2*fdo-1), gc0 = g*(1-fdo);
                        # gf_b = sigmoid(2*fs*ft)*gc1 + gc0
                        c1 = ppp.tile([128, w], dt.float32, tag="c1")
                        nc.vector.tensor_scalar(
                            c1[:], fdoh[:], 2.0, -1.0, alu.mult, alu.add)
                        nc.vector.tensor_mul(c1[:], c1[:], g[:])
                        c0 = ppp.tile([128, w], dt.float32, tag="c0")
                        nc.vector.tensor_scalar(
                            c0[:], fdoh[:], -1.0, 1.0, alu.mult, alu.add)
                        nc.vector.tensor_mul(c0[:], c0[:], g[:])
                        for b, (fsd, ftd) in enumerate(
                                ((fs0_d, ft0_d), (fs1_d, ft1_d))):
                            fs = ppp.tile([128, w], dt.bfloat16, tag=f"fs{b}")
                            ft = ppp.tile([128, w], dt.bfloat16, tag=f"ft{b}")
                            nc.sync.dma_start(fs[:], fsd[:, sl])
                            nc.sync.dma_start(ft[:], ftd[:, sl])
                            z = ppp.tile([128, w], dt.float32, tag=f"z{b}")
                            nc.vector.tensor_mul(z[:], fs[:], ft[:])
                            nc.scalar.activation(z[:], z[:], act.Sigmoid,
                                                 scale=2.0)
                            nc.vector.tensor_mul(z[:], z[:], c1[:])
                            nc.vector.tensor_tensor(
                                gall3[:, sl, 1 + b], z[:], c0[:], alu.add)

                def do_span(si):
                    t0, t1, c0, c1 = spans[si]
                    L = c1 - c0
                    gs = t1 - t0
                    offs = cfg.span_offs(si)
                    OW = offs[-1]

                    xgs = xg_pre.pop(si)
                    issue_xg(si + PF)

                    # one dst tile per scatter call: local_scatter requires
                    # an offset-0 contiguous destination
                    ohx_of = {}
                    for ci, (k0, k1, io, ni) in enumerate(span_calls[si]):
                        ot = ohxp.tile([128, offs[k1] - offs[k0]],
                                       dt.bfloat16, tag=f"ohx{ci}")
                        nc.gpsimd.local_scatter(
                            ot[:],
                            gall_sb[:, 3 * (c0 + k0):3 * (c0 + k0) + ni],
                            idxs_sb[:, io:io + ni],
                            channels=128, num_elems=offs[k1] - offs[k0],
                            num_idxs=ni)
                        for k in range(k0, k1):
                            ohx_of[k] = (ot, offs[k0])

                    ps = pstv.tile([128, gs * T3], dt.float32, tag="ps")
                    nc.tensor.matmul(
                        out=ps[:], lhsT=zero_sb[:, :CHUNK],
                        rhs=zero_sb[:, :gs * T3],
                        start=True, stop=False, skip_group_check=True)
                    for k in range(L):
                        a_k, w_k = cfg.wks[si][k]
                        ot, ob = ohx_of[k]
                        nc.tensor.matmul(
                            out=ps[:, a_k * T3:(a_k + w_k) * T3],
                            lhsT=xgs[:, k * CHUNK:(k + 1) * CHUNK],
                            rhs=ot[:, offs[k] - ob:offs[k + 1] - ob],
                            start=False, stop=(k == L - 1),
                            skip_group_check=True,
                        )
                    # psum -> global accum buffers (cast bf16), on ACT
                    ps4 = ps[:].rearrange("p (g q t) -> p g q t", q=3, t=T)
                    for q in range(3):
                        nc.scalar.activation(
                            vq_sb[q][:, t0 * T:t1 * T]
                            .rearrange("p (g t) -> p g t", t=T),
                            ps4[:, :, q, :], act.Copy)

                def do_window(w, bi):
                    rows = slice(C * bi, C * bi + C)
                    ws = slice(w * WIN, (w + 1) * WIN)
                    vb = vq_sb[1 + bi]
                    op = pso.tile([WIN, C], dt.float32, tag=f"op{bi}")
                    nc.tensor.matmul(out=op[:], lhsT=um_sb[rows, ws],
                                     rhs=wct_sb[rows, :],
                                     start=True, stop=False)
                    nc.tensor.matmul(out=op[:], lhsT=vb[rows, ws],
                                     rhs=wdl_sb[rows, :],
                                     start=False, stop=True)
                    o_sb = outsp.tile([WIN, C], dt.float32, tag=f"os{bi}")
                    nc.scalar.activation(o_sb[:], op[:], act.Copy,
                                         scale=disown_sb[:WIN, w:w + 1])
                    if cfg.has_bias:
                        nc.vector.tensor_add(o_sb[:], o_sb[:],
                                             biasf_sb[:WIN, :])
                    nc.sync.dma_start(outs[bi][ws, :], o_sb[:])

                # interleave: issue epilogue windows as their tiles complete
                nwin_done = 0
                for si in range(len(spans)):
                    do_span(si)
                    ready_nodes = spans[si][1] * T
                    while (nwin_done < nwin
                           and (nwin_done + 1) * WIN <= ready_nodes):
                        for bi in range(2):
                            do_window(nwin_done, bi)
                        nwin_done += 1
                while nwin_done < nwin:
                    for bi in range(2):
                        do_window(nwin_done, bi)
                    nwin_done += 1

    nc.compile()
    return nc


def _shared_weights(W_conc, W_disc, bias):
    wct2 = np.zeros((128, C), np.float32)
    wdt2 = np.zeros((128, C), np.float32)
    wct2[:C] = np.asarray(W_conc, np.float32).T  # WcT[i, o] = Wc[o, i]
    wct2[C:] = wct2[:C]
    wdt2[:C] = np.asarray(W_disc, np.float32).T
    wdt2[C:] = wdt2[:C]
    biasr = np.tile(np.asarray(bias, np.float32)[None, :], (128, 1))
    return wct2, wdt2, biasr


_NC_CACHE = {}


def _run(inputs, trace=False):
    from concourse.bass_utils import run_bass_kernel_spmd

    x = np.asarray(inputs["x"], np.float32)
    n = x.shape[1]
    src_s, tgt_s, pe_s, deg, xpack = _edge_meta(
        x, inputs["edge_index"], inputs["f_disc_orig"], inputs["fluxes"], n)
    scs, wks = _span_meta(tgt_s, TILE, n, N_CORES)
    cfg = Cfg(n_nodes=n, n_cores=N_CORES, tile=TILE, scs=scs, wks=wks,
              has_bias=bool(np.any(np.asarray(inputs["bias"]))))

    wct2, wdt2, biasr = _shared_weights(
        inputs["W_conc"], inputs["W_disc"], inputs["bias"])

    in_maps = []
    for core in range(cfg.n_cores):
        m = prep_core(core, cfg, src_s, tgt_s, pe_s, deg, xpack)
        m.update(wct2=wct2, wdt2=wdt2, biasr=biasr)
        in_maps.append(m)

    if cfg not in _NC_CACHE:
        _NC_CACHE[cfg] = build_nc(cfg)
    nc = _NC_CACHE[cfg]

    res = run_bass_kernel_spmd(nc, in_maps, list(range(cfg.n_cores)),
                               trace=trace)
    out = np.zeros((BATCH, n, C), np.float32)
    npc = cfg.npc
    for core in range(cfg.n_cores):
        out[0, core * npc:(core + 1) * npc] = res.results[core]["out0"][:npc]
        out[1, core * npc:(core + 1) * npc] = res.results[core]["out1"][:npc]
    return out, res


def kernel(x, edge_index, f_disc_orig, fluxes, W_conc, W_disc, bias):
    out, _ = _run(dict(x=x, edge_index=edge_index, f_disc_orig=f_disc_orig,
                       fluxes=fluxes, W_conc=W_conc, W_disc=W_disc, bias=bias))
    return out


def profile_run(inputs):
    out, res = _run(inputs, trace=True)
    return res.exec_time_ns
